# revision 1
# baseline (speedup 1.0000x reference)
"""DirGATConv on 8 Trainium2 NeuronCores (Bass/Tile).

Strategy (node/data parallel, no collectives):
  - Each core owns 6250 destination nodes (a contiguous range).
  - Phase A (replicated): compute h = x@W per direction for all nodes plus
    per-node attention projections es = x@(W a_src), ed = x@(W a_dst), and
    write gather tables to DRAM. Table row (320 fp32 = 1280 B):
      [h_head0(64) | 1.0 | h_head1 | 1.0 | h_head2 | 1.0 | h_head3 | 1.0 |
       es(4) | pad]
    The interleaved 1.0 columns make each head's mask-matmul accumulate the
    softmax denominator in the same PSUM tile as the numerator.
    Tables are split into two row-banks (25088 / 24960) because dma_gather
    indices are int16. A per-core local table holds ed for the core's own
    destinations (indices are dst - core*6250 < 6250).
  - Phase B: destinations are permuted into 51 blocks of <=128 nodes by a
    4-constraint bin-packing so each (block, direction, src-bank) needs at
    most 5 chunks of 128 edges. Per chunk: gather source rows, build 4
    per-head masks  maskp_h[e,d] = (dst_local[e]==d) * p[e,h]  with one fused
    tensor_scalar each, and matmul  maskp_h^T @ rows  into PSUM.
    p = exp(leaky_relu(es[src]+ed[dst])); softmax normalization is applied
    after aggregation (numerator and denominator are both linear in p).
"""

import numpy as np

import concourse.bacc as bacc
import concourse.mybir as mybir
import concourse.tile as tile
from concourse.bass_utils import run_bass_kernel_spmd
from concourse import library_config

# problem constants
N, E, DIN, H, C = 50000, 400000, 256, 4, 64
HC = H * C
ALPHA, SLOPE = 0.5, 0.2

# distribution constants
NCORES = 8
NPC = N // NCORES              # 6250 destinations per core
BANK0 = 25088                  # src-table bank split (196 tiles of 128)
BANK1 = N + 48 - BANK0         # 24960 (195 tiles); tables padded to 50048 rows
NT0, NT1 = BANK0 // 128, BANK1 // 128
NTILE = NT0 + NT1              # 391
NBIN = 51                      # destination blocks per core
CB = 5                         # chunks per (block, src-bank)
CPB = 2 * CB                   # chunks per block
NLOCT = 49                     # local tiles (49*128 = 6272 >= 6250)
NLOC = NLOCT * 128
TW = 320                       # table row width (floats); 1280 B
LW = 64                        # local table row width; 256 B
F32 = mybir.dt.float32
I16 = mybir.dt.int16


def build_kernel(num_swdge_queues=1, gather_queues=1, mode="full", nbin=NBIN):
    nc = bacc.Bacc("TRN2", num_swdge_queues=num_swdge_queues)

    x = nc.dram_tensor("x", [N, DIN], F32, kind="ExternalInput")
    x_loc = nc.dram_tensor("x_loc", [NLOC, DIN], F32, kind="ExternalInput")
    W1 = nc.dram_tensor("W1", [DIN, HC], F32, kind="ExternalInput")
    W2 = nc.dram_tensor("W2", [DIN, HC], F32, kind="ExternalInput")
    a_in = nc.dram_tensor("a_in", [4, H, C, 1], F32, kind="ExternalInput")  # src1,dst1,src2,dst2
    b_in = nc.dram_tensor("b_in", [1, HC], F32, kind="ExternalInput")    # 0.5*(b1+b2)
    iota_in = nc.dram_tensor("iota_in", [128, 128], F32, kind="ExternalInput")
    ident_in = nc.dram_tensor("ident_in", [128, 128], F32, kind="ExternalInput")
    gidx = nc.dram_tensor("gidx", [2, NBIN, 128, 160], I16, kind="ExternalInput")
    dcol = nc.dram_tensor("dcol", [2, NBIN, 128, CPB], F32, kind="ExternalInput")
    out = nc.dram_tensor("out", [NBIN * 128, HC], F32, kind="ExternalOutput")

    with tile.TileContext(nc) as tc:
        with (
            tc.tile_pool(name="dram", bufs=1, space="DRAM") as dpool,
            tc.tile_pool(name="const", bufs=1) as cpool,
        ):
            nc.gpsimd.load_library(library_config.mlp)

            tabs = [
                [dpool.tile([BANK0, TW], F32, tag=f"tab{d}0", name=f"tab{d}0"),
                 dpool.tile([BANK1, TW], F32, tag=f"tab{d}1", name=f"tab{d}1")]
                for d in range(2)
            ]
            loctab = dpool.tile([NLOC, LW], F32, tag="loctab")

            iota_t = cpool.tile([128, 128], F32)
            nc.sync.dma_start(iota_t[:], iota_in[:])
            ident_t = cpool.tile([128, 128], F32)
            nc.sync.dma_start(ident_t[:], ident_in[:])

            # weights: W_sb[d][k] = W_{d+1}[k*128:(k+1)*128, :]   [128 din, 256]
            W_sb = [[cpool.tile([128, HC], F32, tag=f"w{d}{k}", name=f"w{d}{k}")
                     for k in range(2)] for d in range(2)]
            for d, wdram in enumerate((W1, W2)):
                for k in range(2):
                    nc.sync.dma_start(W_sb[d][k][:], wdram[k * 128:(k + 1) * 128, :])

            # A matrices: A[d][kc]  [128 hc, 8]  block-diag of (a_src_d | a_dst_d)
            A_sb = [cpool.tile([128, 2, 8], F32, tag=f"a{d}", name=f"a{d}")
                    for d in range(2)]
            for d in range(2):
                nc.vector.memset(A_sb[d][:], 0.0)
                for j in range(2):          # 0: a_src, 1: a_dst
                    for h in range(H):
                        hc0 = h * C
                        kc, off = divmod(hc0, 128)
                        jj = j * H + h
                        nc.sync.dma_start(
                            A_sb[d][off:off + C, kc, jj:jj + 1],
                            a_in[2 * d + j, h, :, :],
                        )

            # WT[d][kc]  [128 hc, 256 din]  via PE transposes
            WT = [cpool.tile([128, 2, 256], F32, tag=f"wt{d}", name=f"wt{d}")
                  for d in range(2)]
            with tc.tile_pool(name="psA0", bufs=2, space="PSUM") as ps0:
                for d in range(2):
                    for kc in range(2):
                        for m in range(2):
                            pt = ps0.tile([128, 128], F32, tag="ptr")
                            nc.tensor.transpose(
                                pt[:], W_sb[d][m][:, kc * 128:(kc + 1) * 128],
                                ident_t[:])
                            nc.vector.tensor_copy(
                                WT[d][:, kc, m * 128:(m + 1) * 128], pt[:])
                # Wsd[k]  [128 din, 16]: cols 0:8 dir1 (es|ed), 8:16 dir2
                wsd = cpool.tile([128, 2, 16], F32)
                for m in range(2):
                    pw = ps0.tile([128, 16], F32, tag="pw")
                    for d in range(2):
                        for kc in range(2):
                            nc.tensor.matmul(
                                pw[:, d * 8:(d + 1) * 8],
                                WT[d][:, kc, m * 128:(m + 1) * 128],
                                A_sb[d][:, kc, :],
                                start=(kc == 0), stop=(kc == 1),
                            )
                    nc.vector.tensor_copy(wsd[:, m, :], pw[:])

                # bias broadcast tile: ones(128,1) x b_in(1,256)
                ones_row = cpool.tile([1, 128], F32)
                nc.vector.memset(ones_row[:], 1.0)
                brow = cpool.tile([1, HC], F32)
                nc.sync.dma_start(brow[:], b_in[:])
                bias_bc = cpool.tile([128, HC], F32)
                pb = ps0.tile([128, HC], F32, tag="pb")
                nc.tensor.matmul(pb[:], ones_row[:], brow[:])
                nc.vector.tensor_copy(bias_bc[:], pb[:])

            # ---------------- Phase A ----------------
            with (
                tc.tile_pool(name="pA", bufs=3) as pa,
                tc.tile_pool(name="pAst", bufs=3) as past,
                tc.tile_pool(name="psA", bufs=2, space="PSUM") as psa,
            ):
                def node_tile(xa_src, write_tabs, write_loc, partial):
                    xt = pa.tile([128, DIN], F32, tag="xt")
                    if partial:
                        nc.vector.memset(xt[:, :], 0.0)
                        nc.sync.dma_start(xt[:partial, :], xa_src)
                    else:
                        nc.sync.dma_start(xt[:], xa_src)
                    pxt = psa.tile([128, 2, 128], F32, tag="pxt")
                    for k in range(2):
                        nc.tensor.transpose(
                            pxt[:, k, :], xt[:, k * 128:(k + 1) * 128], ident_t[:])
                    xts = pa.tile([128, 2, 128], F32, tag="xts")
                    nc.vector.tensor_copy(xts[:], pxt[:])

                    pes = psa.tile([128, 16], F32, tag="pes")
                    for k in range(2):
                        nc.tensor.matmul(pes[:], xts[:, k, :], wsd[:, k, :],
                                         start=(k == 0), stop=(k == 1))
                    if write_tabs is not None:
                        for d in range(2):
                            ph = psa.tile([128, HC], F32, tag=f"ph{d}")
                            for k in range(2):
                                nc.tensor.matmul(
                                    ph[:], xts[:, k, :], W_sb[d][k][:],
                                    start=(k == 0), stop=(k == 1))
                            st = past.tile([128, TW], F32, tag=f"st{d}")
                            stv = st[:, 0:260].rearrange("p (h w) -> p h w", w=65)
                            nc.vector.tensor_copy(
                                stv[:, :, 0:64],
                                ph[:].rearrange("p (h w) -> p h w", w=64))
                            nc.vector.memset(stv[:, :, 64], 1.0)
                            nc.vector.tensor_copy(st[:, 260:264], pes[:, d * 8:d * 8 + 4])
                            nc.vector.memset(st[:, 264:TW], 0.0)
                            nc.sync.dma_start(write_tabs[d], st[:])
                    if write_loc is not None:
                        stl = past.tile([128, LW], F32, tag="stl")
                        nc.vector.tensor_copy(stl[:, 0:4], pes[:, 4:8])
                        nc.vector.tensor_copy(stl[:, 4:8], pes[:, 12:16])
                        nc.vector.memset(stl[:, 8:LW], 0.0)
                        nc.sync.dma_start(write_loc, stl[:])

                for t in range(NTILE):
                    partial = 80 if t == NTILE - 1 else 0
                    rows = x[t * 128: min((t + 1) * 128, N), :]
                    bk = 0 if t < NT0 else 1
                    r0 = t * 128 - (0 if bk == 0 else BANK0)
                    wt = [tabs[d][bk][r0:r0 + 128, :] for d in range(2)]
                    node_tile(rows, wt, None, partial)
                for t in range(NLOCT):
                    rows = x_loc[t * 128:(t + 1) * 128, :]
                    node_tile(rows, None, loctab[t * 128:(t + 1) * 128, :], 0)

            # ---------------- Phase B ----------------
            with (
                tc.tile_pool(name="pBg", bufs=2) as pg,
                tc.tile_pool(name="pBm", bufs=3) as pm,
                tc.tile_pool(name="pBmask", bufs=8) as pmask,
                tc.tile_pool(name="pBo", bufs=2) as po,
                tc.tile_pool(name="psB", bufs=4, space="PSUM") as psb,
            ):
                for b in range(0 if mode == "A" else nbin):
                    stage = [None, None]
                    for d in range(2):
                        gi = pm.tile([128, 160], I16, tag="gi")
                        nc.sync.dma_start(gi[:], gidx[d, b, :, :])
                        dc = pm.tile([128, CPB], F32, tag="dc")
                        nc.sync.dma_start(dc[:], dcol[d, b, :, :])

                        srcg = pg.tile([128, CPB, TW], F32, tag="srcg")
                        qq = (2 * b + d) % gather_queues
                        nc.gpsimd.dma_gather(
                            srcg[:, 0:CB, :], tabs[d][0][:], gi[:, 0:40],
                            CB * 128, CB * 128, TW, queue_num=qq)
                        nc.gpsimd.dma_gather(
                            srcg[:, CB:CPB, :], tabs[d][1][:], gi[:, 40:80],
                            CB * 128, CB * 128, TW, queue_num=qq)
                        tailg = pg.tile([128, CPB, LW], F32, tag="tailg")
                        nc.gpsimd.dma_gather(
                            tailg[:, 0:CB, :], loctab[:], gi[:, 80:120],
                            CB * 128, CB * 128, LW, queue_num=qq)
                        nc.gpsimd.dma_gather(
                            tailg[:, CB:CPB, :], loctab[:], gi[:, 120:160],
                            CB * 128, CB * 128, LW, queue_num=qq)

                        if mode == "G":
                            if d == 0:
                                nc.sync.dma_start(
                                    out[b * 128:(b + 1) * 128, :],
                                    srcg[:, 0, 0:HC])
                            continue

                        # p = exp(leaky_relu(es_src + ed_dst))  [128, CPB, 4]
                        lg = pm.tile([128, CPB, 4], F32, tag="lg")
                        nc.vector.tensor_tensor(
                            lg[:], srcg[:, :, 260:264],
                            tailg[:, :, 4 * d:4 * d + 4], mybir.AluOpType.add)
                        ls = pm.tile([128, CPB, 4], F32, tag="ls")
                        nc.vector.tensor_scalar(
                            out=ls[:], in0=lg[:], scalar1=SLOPE, scalar2=None,
                            op0=mybir.AluOpType.mult)
                        nc.vector.tensor_tensor(lg[:], lg[:], ls[:],
                                                mybir.AluOpType.max)
                        pt = pm.tile([128, CPB, 4], F32, tag="pt")
                        nc.scalar.activation(pt[:], lg[:],
                                             mybir.ActivationFunctionType.Exp)

                        pnd = psb.tile([128, H, 65], F32, tag="pnd")
                        for h in range(H):
                            for cc in range(CPB):
                                mp = pmask.tile([128, 128], F32, tag="mp")
                                nc.vector.tensor_scalar(
                                    out=mp[:], in0=iota_t[:],
                                    scalar1=dc[:, cc:cc + 1],
                                    scalar2=pt[:, cc, h:h + 1],
                                    op0=mybir.AluOpType.is_equal,
                                    op1=mybir.AluOpType.mult)
                                nc.tensor.matmul(
                                    pnd[:, h, :], mp[:], srcg[:, cc, 65 * h:65 * h + 65],
                                    start=(cc == 0), stop=(cc == CPB - 1))

                        # stage_d[:, h*64:(h+1)*64] = num_h / (2*den_h)
                        d2 = pm.tile([128, H], F32, tag="d2")
                        nc.vector.tensor_scalar(
                            out=d2[:], in0=pnd[:, :, 64], scalar1=2.0, scalar2=1e-9,
                            op0=mybir.AluOpType.mult, op1=mybir.AluOpType.max)
                        rec = pm.tile([128, H], F32, tag="rec")
                        nc.vector.reciprocal(rec[:], d2[:])
                        stage[d] = po.tile([128, HC], F32, tag=f"stage{d}", name=f"stage{d}")
                        for h in range(H):
                            nc.vector.tensor_scalar(
                                out=stage[d][:, h * 64:(h + 1) * 64],
                                in0=pnd[:, h, 0:64],
                                scalar1=rec[:, h:h + 1], scalar2=None,
                                op0=mybir.AluOpType.mult)

                    if mode == "G":
                        continue
                    ot = po.tile([128, HC], F32, tag="ot")
                    nc.vector.tensor_tensor(ot[:], stage[0][:], stage[1][:],
                                            mybir.AluOpType.add)
                    nc.vector.tensor_tensor(ot[:], ot[:], bias_bc[:],
                                            mybir.AluOpType.add)
                    nc.sync.dma_start(out[b * 128:(b + 1) * 128, :], ot[:])

    nc.compile()
    return nc


# ---------------------------------------------------------------- host side

def _wrap16(arr):
    """int idx array [n] -> dma_gather layout [128, n/16] int16 (replicated)."""
    n = len(arr)
    m = arr.reshape(n // 16, 16).astype(np.int16).T  # [16, n/16]
    return np.tile(m, (8, 1))


def prep_inputs(x, edge_index, W1, a_src1, a_dst1, b1, W2, a_src2, a_dst2, b2):
    x = np.asarray(x, np.float32)
    ei = np.asarray(edge_index)
    src, dst = ei[0].astype(np.int64), ei[1].astype(np.int64)
    loops = np.arange(N, dtype=np.int64)
    dirs = [
        (np.concatenate([src, loops]), np.concatenate([dst, loops])),
        (np.concatenate([dst, loops]), np.concatenate([src, loops])),
    ]

    # per-node degree by (dir, src-bank)
    deg = np.zeros((N, 4), np.int64)
    for j, (ss, dd) in enumerate(dirs):
        for bk in range(2):
            m = (ss >= BANK0) == (bk == 1)
            deg[:, 2 * j + bk] = np.bincount(dd[m], minlength=N)

    iota_in = np.broadcast_to(np.arange(128, dtype=np.float32), (128, 128)).copy()
    ident_in = np.eye(128, dtype=np.float32)
    a_in = np.stack([a_src1, a_dst1, a_src2, a_dst2]).astype(np.float32)[..., None]
    b_in = (0.5 * (np.asarray(b1) + np.asarray(b2))).astype(np.float32).reshape(1, HC)

    in_maps, perms = [], []
    for core in range(NCORES):
        lo = core * NPC
        nodes = np.arange(lo, lo + NPC)
        order = nodes[np.argsort(-deg[nodes].sum(1), kind="stable")]
        degs = deg[order]
        bins_load = np.zeros((NBIN, 4), np.int64)
        bins_cnt = np.zeros(NBIN, np.int64)
        node_blk = np.full(N, -1, np.int64)
        node_slot = np.full(N, -1, np.int64)
        for i_n in range(len(order)):
            dgl = degs[i_n]
            ok = (bins_cnt < 128) & ((bins_load + dgl) <= CB * 128).all(1)
            assert ok.any(), "bin packing failed; raise NBIN/CB"
            cand = np.where(ok)[0]
            nl = (bins_load[cand] + dgl).max(1) * 1000 + bins_cnt[cand]
            i = cand[np.argmin(nl)]
            node_blk[order[i_n]] = i
            node_slot[order[i_n]] = bins_cnt[i]
            bins_load[i] += dgl
            bins_cnt[i] += 1

        perm = np.full(NBIN * 128, -1, np.int64)
        perm[node_blk[nodes] * 128 + node_slot[nodes]] = nodes
        perms.append(perm)

        g_idx = np.zeros((2, NBIN, 128, 160), np.int16)
        d_col = np.full((2, NBIN, 128, CPB), -1.0, np.float32)
        for d, (ss, dd) in enumerate(dirs):
            sel = (dd >= lo) & (dd < lo + NPC)
            es_, ed_ = ss[sel], dd[sel]
            blk = node_blk[ed_]
            bank = (es_ >= BANK0).astype(np.int64)
            eo = np.lexsort((bank, blk))
            es_, ed_, blk, bank = es_[eo], ed_[eo], blk[eo], bank[eo]
            # slot position within (blk, bank) segment
            seg = blk * 2 + bank
            segbnd = np.flatnonzero(np.diff(seg, prepend=-1))
            within = np.arange(len(seg)) - np.repeat(segbnd, np.diff(
                np.append(segbnd, len(seg))))
            assert (within < CB * 128).all()
            slot = within + np.where(bank == 0, 0, CB * 128)
            srcrel = np.where(bank == 0, es_, es_ - BANK0)
            s_idx = np.zeros((NBIN, CPB * 128), np.int64)
            t_idx = np.zeros((NBIN, CPB * 128), np.int64)
            dloc = np.full((NBIN, CPB * 128), -1.0, np.float32)
            s_idx[blk, slot] = srcrel
            t_idx[blk, slot] = ed_ - lo
            dloc[blk, slot] = node_slot[ed_]
            for b in range(NBIN):
                g_idx[d, b, :, 0:40] = _wrap16(s_idx[b, 0:CB * 128])
                g_idx[d, b, :, 40:80] = _wrap16(s_idx[b, CB * 128:])
                g_idx[d, b, :, 80:160] = _wrap16(t_idx[b])
                d_col[d, b] = dloc[b].reshape(CPB, 128).T

        x_loc = np.zeros((NLOC, DIN), np.float32)
        x_loc[:NPC] = x[lo:lo + NPC]
        in_maps.append({
            "x": x, "x_loc": x_loc,
            "W1": np.asarray(W1, np.float32), "W2": np.asarray(W2, np.float32),
            "a_in": a_in, "b_in": b_in,
            "iota_in": iota_in, "ident_in": ident_in,
            "gidx": g_idx, "dcol": d_col,
        })
    return in_maps, perms


_NC_CACHE = {}


def kernel(**inputs):
    in_maps, perms = prep_inputs(**inputs)
    key = "k1"
    if key not in _NC_CACHE:
        _NC_CACHE[key] = build_kernel()
    nc = _NC_CACHE[key]
    res = run_bass_kernel_spmd(nc, in_maps, list(range(NCORES)))
    result = np.empty((N, HC), np.float32)
    for core in range(NCORES):
        o = res.results[core]["out"]
        p = perms[core]
        valid = p >= 0
        result[p[valid]] = o[valid]
    return result



# revision 16
# speedup vs baseline: 2.7671x; 2.7671x over previous
"""DirGATConv on 8 Trainium2 NeuronCores (Bass/Tile), v2 direct-gather fp16.

Strategy (node/data parallel, no collectives):
  - Each core owns 6250 destination nodes, bin-packed into 51 blocks of <=128
    so each (block, direction, src-bank) needs at most 5 chunks of 128 edges.
  - x is passed as two fp16 row banks (512-B rows, int16 gather indices).
    Per (block, dir, bank) one TRANSPOSE-mode dma_gather fetches the source
    rows already transposed: xg[p, k, i] = x[src_i][k*128+p], ready to be used
    as PE weights (lhsT) for the per-chunk projection.
  - Per chunk (128 edges): project h = x_src @ W_d (fp16, 2 k-chunks) and
    es = x_src @ w_es_d riding the same loaded weights (ldweights=False).
    ed[dst] is fetched via a tiny matmul with the host-built one-hot mask
    transpose MT: ed_c = MT^T @ ed_blk.  p = exp(leakyrelu(es+ed)) is computed
    batched per (block, dir): DVE add + fused max(x, 0.2x), exp on Scalar.
  - Aggregation: one fp16 matmul per chunk with the host-built 0/1 mask M as
    stationary weights: agg[:, 0:256] += M^T @ (h * p_bcast),
    agg[:, 256:260] += M^T @ p (softmax denominators), second matmul reuses
    the loaded mask weights (ldweights=False).
  - Softmax normalization after aggregation (numerator and denominator are
    both linear in p); combine directions with alpha=0.5 and add bias.
"""

import numpy as np

import concourse.bacc as bacc
import concourse.mybir as mybir
import concourse.tile as tile
from concourse.bass_utils import run_bass_kernel_spmd
from concourse import library_config

# problem constants
N, E, DIN, H, C = 50000, 400000, 256, 4, 64
HC = H * C
ALPHA, SLOPE = 0.5, 0.2

# distribution constants
NCORES = 8
NPC = N // NCORES              # 6250 destinations per core
BANK0 = 25088                  # x-bank split (int16 gather indices)
BANK1 = N + 48 - BANK0         # 24960; x padded to 50048 rows
NBIN = 51                      # destination blocks per core
CB = 5                         # chunks per (block, src-bank)
CPB = 2 * CB                   # chunks per block
NLOC = NBIN * 128              # 6528 local slots (perm order)
F32 = mybir.dt.float32
F16 = mybir.dt.float16
I16 = mybir.dt.int16

# z-scale engine per chunk parity: 'v' = DVE, 'g' = GpSimd
Z_ENG = "v"


def build_kernel():
    nc = bacc.Bacc("TRN2", num_swdge_queues=4)

    xb0 = nc.dram_tensor("xb0", [BANK0, DIN], F16, kind="ExternalInput")
    xb1 = nc.dram_tensor("xb1", [BANK1, DIN], F16, kind="ExternalInput")
    xtl = nc.dram_tensor("xtl", [DIN, NLOC], F16, kind="ExternalInput")
    wh = nc.dram_tensor("wh", [2, 2, 128, HC], F16, kind="ExternalInput")
    wes = nc.dram_tensor("wes", [2, 2, 128, H], F16, kind="ExternalInput")
    wed = nc.dram_tensor("wed", [2, 128, 2 * H], F16, kind="ExternalInput")
    bias = nc.dram_tensor("bias", [128, HC], F32, kind="ExternalInput")
    gidx = nc.dram_tensor("gidx", [2, NBIN, 128, 80], I16, kind="ExternalInput")
    mks = nc.dram_tensor("mks", [2, NBIN, 128, CPB, 128], F16,
                         kind="ExternalInput")
    mkt = nc.dram_tensor("mkt", [2, NBIN, 128, CPB, 128], F16,
                         kind="ExternalInput")
    out = nc.dram_tensor("out", [NLOC, HC], F32, kind="ExternalOutput")

    with tile.TileContext(nc) as tc:
        with tc.tile_pool(name="const", bufs=1) as cp:
            nc.gpsimd.load_library(library_config.mlp)

            # weights
            wh_sb = [cp.tile([128, 2, HC], F16, tag=f"wh{d}", name=f"wh{d}")
                     for d in range(2)]
            wes_sb = [cp.tile([128, 2, H], F16, tag=f"wes{d}", name=f"wes{d}")
                      for d in range(2)]
            for d in range(2):
                for k in range(2):
                    nc.sync.dma_start(wh_sb[d][:, k, :], wh[d, k, :, :])
                    nc.sync.dma_start(wes_sb[d][:, k, :], wes[d, k, :, :])
            wed_sb = cp.tile([128, 2, 2 * H], F16)
            for k in range(2):
                nc.sync.dma_start(wed_sb[:, k, :], wed[k, :, :])
            bias_sb = cp.tile([128, HC], F32)
            nc.sync.dma_start(bias_sb[:], bias[:])

            # ---------------- Phase A-lite: ed for local (permuted) nodes ---
            ed_sb = cp.tile([128, NBIN, 2 * H], F16, name="ed_sb")
            with (
                tc.tile_pool(name="pA", bufs=3) as pa,
                tc.tile_pool(name="psA", bufs=2, space="PSUM") as psa,
            ):
                for t in range(NBIN):
                    xlt = pa.tile([128, 2, 128], F16, tag="xlt")
                    for k in range(2):
                        nc.sync.dma_start(
                            xlt[:, k, :],
                            xtl[k * 128:(k + 1) * 128, t * 128:(t + 1) * 128])
                    ped = psa.tile([128, 2 * H], F32, tag="ped")
                    for k in range(2):
                        nc.tensor.matmul(ped[:], xlt[:, k, :], wed_sb[:, k, :],
                                         start=(k == 0), stop=(k == 1))
                    nc.vector.tensor_copy(ed_sb[:, t, :], ped[:])

            # ---------------- Phase B ----------------
            with (
                tc.tile_pool(name="pBg", bufs=3) as pg,
                tc.tile_pool(name="pBm", bufs=3) as pm,
                tc.tile_pool(name="pBs", bufs=4) as psb,
                tc.tile_pool(name="pBo", bufs=3) as po,
                tc.tile_pool(name="psH", bufs=3, space="PSUM") as psh,
                tc.tile_pool(name="psG", bufs=2, space="PSUM") as psg,
            ):
                for b in range(NBIN):
                    stage = [None, None]
                    for d in range(2):
                        gi = pm.tile([128, 80], I16, tag="gi")
                        nc.sync.dma_start(gi[:], gidx[d, b, :, :])
                        xg = [pg.tile([128, 2, CB * 128], F16, tag=f"xg{bk}",
                                      name=f"xg{bk}") for bk in range(2)]
                        qq = (2 * b + d) * 2
                        nc.gpsimd.dma_gather(
                            xg[0][:], xb0[:], gi[:, 0:40], CB * 128, CB * 128,
                            DIN, transpose=True, queue_num=qq % 4)
                        nc.gpsimd.dma_gather(
                            xg[1][:], xb1[:], gi[:, 40:80], CB * 128, CB * 128,
                            DIN, transpose=True, queue_num=(qq + 1) % 4)
                        mks_t = pm.tile([128, CPB, 128], F16, tag="mks")
                        nc.sync.dma_start(mks_t[:], mks[d, b, :, :, :])
                        mkt_t = pm.tile([128, CPB, 128], F16, tag="mkt")
                        nc.sync.dma_start(mkt_t[:], mkt[d, b, :, :, :])

                        # PSUM: num bank [0:256]; aux bank: esp[0:40] |
                        # edp[40:80] | den1[80:84] | den2[84:88].
                        # HW rule: only ONE accumulation group may be open per
                        # bank at a time; groups below are sequenced per bank.
                        num = psg.tile([128, 512], F32, tag="num")
                        aux = psg.tile([128, 512], F32, tag="aux")
                        esp = aux[:, 0:40].rearrange("p (a b) -> p a b", b=H)
                        edp = aux[:, 40:80].rearrange("p (a b) -> p a b", b=H)

                        # ed per chunk: ed_c = MT^T @ ed_blk
                        for cc in range(CPB):
                            nc.tensor.matmul(
                                edp[:, cc, :], mkt_t[:, cc, :],
                                ed_sb[:, b, H * d:H * d + H],
                                start=True, stop=True)

                        pt = psb.tile([128, CPB, H], F16, tag="pt")
                        HB = CPB // 2
                        for half in range(2):
                            ccs = range(half * HB, (half + 1) * HB)
                            hsl = slice(half * HB, (half + 1) * HB)
                            # projection; es batched into esp
                            hps = {}
                            for cc in ccs:
                                bk, c0 = divmod(cc, CB)
                                sl = slice(c0 * 128, (c0 + 1) * 128)
                                j = cc - half * HB
                                if j % 2 == 0:
                                    hpair = psh.tile([128, 2, HC], F32,
                                                     tag="hp")
                                hp = hpair[:, j % 2, :]
                                hps[cc] = hp
                                for k in range(2):
                                    nc.tensor.matmul(
                                        hp, xg[bk][:, k, sl],
                                        wh_sb[d][:, k, :],
                                        start=(k == 0), stop=(k == 1))
                                    nc.tensor.matmul(
                                        esp[:, cc, :], xg[bk][:, k, sl],
                                        wes_sb[d][:, k, :],
                                        start=(k == 0), stop=(k == 1))

                            # p = exp(leakyrelu(es + ed)) for this half
                            edv = psb.tile([128, HB, H], F32, tag="edv")
                            nc.vector.tensor_copy(edv[:], edp[:, hsl, :])
                            lg = psb.tile([128, HB, H], F32, tag="lg")
                            nc.vector.tensor_tensor(lg[:], esp[:, hsl, :],
                                                    edv[:],
                                                    mybir.AluOpType.add)
                            lr = psb.tile([128, HB, H], F32, tag="lr")
                            nc.vector.scalar_tensor_tensor(
                                out=lr[:], in0=lg[:], scalar=SLOPE, in1=lg[:],
                                op0=mybir.AluOpType.mult,
                                op1=mybir.AluOpType.max)
                            nc.scalar.activation(
                                pt[:, hsl, :], lr[:],
                                mybir.ActivationFunctionType.Exp)

                            # z = h * p_bcast; aggregate with mask weights
                            dreg = aux[:, 80 + 4 * half:84 + 4 * half]
                            for cc in ccs:
                                j = cc - half * HB
                                z = psb.tile([128, HC], F16, tag="z")
                                nc.vector.tensor_tensor(
                                    z[:].rearrange("p (h c) -> p h c", h=H),
                                    hps[cc].rearrange("p (h c) -> p h c", h=H),
                                    pt[:, cc, :].unsqueeze(2).broadcast_to(
                                        [128, H, C]),
                                    mybir.AluOpType.mult)
                                nc.tensor.matmul(num[:, 0:HC],
                                                 mks_t[:, cc, :], z[:],
                                                 start=(cc == 0),
                                                 stop=(cc == CPB - 1))
                                nc.tensor.matmul(dreg, mks_t[:, cc, :],
                                                 pt[:, cc, :],
                                                 start=(j == 0),
                                                 stop=(j == HB - 1))
                        # normalize: stage = num / den  (den>0: self-loop)
                        dv2 = po.tile([128, H], F32, tag="dv2")
                        nc.vector.tensor_copy(dv2[:], aux[:, 84:88])
                        den = po.tile([128, H], F32, tag="den")
                        nc.vector.tensor_tensor(den[:], aux[:, 80:84],
                                                dv2[:], mybir.AluOpType.add)
                        den2 = po.tile([128, H], F32, tag="den2")
                        nc.vector.tensor_scalar(
                            out=den2[:], in0=den[:], scalar1=2.0,
                            scalar2=1e-12, op0=mybir.AluOpType.mult,
                            op1=mybir.AluOpType.max)
                        rec = po.tile([128, H], F32, tag="rec")
                        nc.vector.reciprocal(rec[:], den2[:])
                        stage[d] = po.tile([128, HC], F32, tag=f"st{d}",
                                           name=f"st{d}")
                        nc.vector.tensor_tensor(
                            stage[d][:].rearrange("p (h c) -> p h c", h=H),
                            num[:, 0:HC].rearrange("p (h c) -> p h c", h=H),
                            rec[:].unsqueeze(2).broadcast_to([128, H, C]),
                            mybir.AluOpType.mult)

                    ot = po.tile([128, HC], F32, tag="ot")
                    nc.gpsimd.tensor_tensor(ot[:], stage[0][:], stage[1][:],
                                            mybir.AluOpType.add)
                    ot2 = po.tile([128, HC], F32, tag="ot2")
                    nc.gpsimd.tensor_tensor(ot2[:], ot[:], bias_sb[:],
                                            mybir.AluOpType.add)
                    nc.sync.dma_start(out[b * 128:(b + 1) * 128, :], ot2[:])

    nc.compile()
    return nc


# ---------------------------------------------------------------- host side

def _wrap16(arr):
    """int idx array [n] -> dma_gather layout [128, n/16] int16 (replicated)."""
    n = len(arr)
    m = arr.reshape(n // 16, 16).astype(np.int16).T  # [16, n/16]
    return np.tile(m, (8, 1))


def prep_inputs(x, edge_index, W1, a_src1, a_dst1, b1, W2, a_src2, a_dst2, b2):
    x = np.asarray(x, np.float32)
    ei = np.asarray(edge_index)
    src, dst = ei[0].astype(np.int64), ei[1].astype(np.int64)
    loops = np.arange(N, dtype=np.int64)
    dirs = [
        (np.concatenate([src, loops]), np.concatenate([dst, loops])),
        (np.concatenate([dst, loops]), np.concatenate([src, loops])),
    ]

    x16 = x.astype(np.float16)
    xpad = np.zeros((BANK0 + BANK1, DIN), np.float16)
    xpad[:N] = x16
    xb0_h, xb1_h = xpad[:BANK0], xpad[BANK0:]

    Ws = [np.asarray(W1, np.float32), np.asarray(W2, np.float32)]
    asrc = [np.asarray(a_src1, np.float32), np.asarray(a_src2, np.float32)]
    adst = [np.asarray(a_dst1, np.float32), np.asarray(a_dst2, np.float32)]
    wh_h = np.zeros((2, 2, 128, HC), np.float16)
    wes_h = np.zeros((2, 2, 128, H), np.float16)
    wed_h = np.zeros((2, 128, 2 * H), np.float16)
    for d in range(2):
        Wd = Ws[d]
        w_es = np.stack([Wd[:, h * C:(h + 1) * C] @ asrc[d][h]
                         for h in range(H)], 1)       # [DIN, H]
        w_ed = np.stack([Wd[:, h * C:(h + 1) * C] @ adst[d][h]
                         for h in range(H)], 1)
        for k in range(2):
            wh_h[d, k] = Wd[k * 128:(k + 1) * 128, :].astype(np.float16)
            wes_h[d, k] = w_es[k * 128:(k + 1) * 128, :].astype(np.float16)
            wed_h[k, :, H * d:H * d + H] = \
                w_ed[k * 128:(k + 1) * 128, :].astype(np.float16)
    bias_h = np.broadcast_to(
        (0.5 * (np.asarray(b1) + np.asarray(b2))).astype(np.float32),
        (128, HC)).copy()

    # per-node degree by (dir, src-bank)
    deg = np.zeros((N, 4), np.int64)
    for j, (ss, dd) in enumerate(dirs):
        for bk in range(2):
            m = (ss >= BANK0) == (bk == 1)
            deg[:, 2 * j + bk] = np.bincount(dd[m], minlength=N)

    in_maps, perms = [], []
    for core in range(NCORES):
        lo = core * NPC
        nodes = np.arange(lo, lo + NPC)
        order = nodes[np.argsort(-deg[nodes].sum(1), kind="stable")]
        degs = deg[order]
        bins_load = np.zeros((NBIN, 4), np.int64)
        bins_cnt = np.zeros(NBIN, np.int64)
        node_blk = np.full(N, -1, np.int64)
        node_slot = np.full(N, -1, np.int64)
        for i_n in range(len(order)):
            dgl = degs[i_n]
            ok = (bins_cnt < 128) & ((bins_load + dgl) <= CB * 128).all(1)
            assert ok.any(), "bin packing failed; raise NBIN/CB"
            cand = np.where(ok)[0]
            nl = (bins_load[cand] + dgl).max(1) * 1000 + bins_cnt[cand]
            i = cand[np.argmin(nl)]
            node_blk[order[i_n]] = i
            node_slot[order[i_n]] = bins_cnt[i]
            bins_load[i] += dgl
            bins_cnt[i] += 1

        perm = np.full(NBIN * 128, -1, np.int64)
        perm[node_blk[nodes] * 128 + node_slot[nodes]] = nodes
        perms.append(perm)

        xtl_h = np.zeros((DIN, NLOC), np.float16)
        valid = perm >= 0
        xtl_h[:, valid] = x16[perm[valid]].T

        g_idx = np.zeros((2, NBIN, 128, 80), np.int16)
        mks_h = np.zeros((2, NBIN, CPB, 128, 128), np.float16)
        mkt_h = np.zeros((2, NBIN, CPB, 128, 128), np.float16)
        for d, (ss, dd) in enumerate(dirs):
            sel = (dd >= lo) & (dd < lo + NPC)
            es_, ed_ = ss[sel], dd[sel]
            blk = node_blk[ed_]
            bank = (es_ >= BANK0).astype(np.int64)
            eo = np.lexsort((bank, blk))
            es_, ed_, blk, bank = es_[eo], ed_[eo], blk[eo], bank[eo]
            seg = blk * 2 + bank
            segbnd = np.flatnonzero(np.diff(seg, prepend=-1))
            within = np.arange(len(seg)) - np.repeat(segbnd, np.diff(
                np.append(segbnd, len(seg))))
            assert (within < CB * 128).all()
            slot = within + np.where(bank == 0, 0, CB * 128)
            srcrel = np.where(bank == 0, es_, es_ - BANK0)
            s_idx = np.zeros((NBIN, CPB * 128), np.int64)
            s_idx[blk, slot] = srcrel
            # one-hot masks: edge at (chunk cc, lane e) -> dst slot
            cc_all = slot // 128
            lane = slot % 128
            dslot = node_slot[ed_]
            mks_h[d, blk, cc_all, lane, dslot] = 1.0
            mkt_h[d, blk, cc_all, dslot, lane] = 1.0
            for bb in range(NBIN):
                g_idx[d, bb, :, 0:40] = _wrap16(s_idx[bb, 0:CB * 128])
                g_idx[d, bb, :, 40:80] = _wrap16(s_idx[bb, CB * 128:])

        in_maps.append({
            "xb0": xb0_h, "xb1": xb1_h, "xtl": xtl_h,
            "wh": wh_h, "wes": wes_h, "wed": wed_h, "bias": bias_h,
            "gidx": g_idx,
            "mks": np.ascontiguousarray(mks_h.transpose(0, 1, 3, 2, 4)),
            "mkt": np.ascontiguousarray(mkt_h.transpose(0, 1, 3, 2, 4)),
        })
    return in_maps, perms


_NC_CACHE = {}


def kernel(**inputs):
    in_maps, perms = prep_inputs(**inputs)
    key = "k2"
    if key not in _NC_CACHE:
        _NC_CACHE[key] = build_kernel()
    nc = _NC_CACHE[key]
    res = run_bass_kernel_spmd(nc, in_maps, list(range(NCORES)))
    result = np.empty((N, HC), np.float32)
    for core in range(NCORES):
        o = res.results[core]["out"]
        p = perms[core]
        valid = p >= 0
        result[p[valid]] = o[valid]
    return result
